# revision 16
# baseline (speedup 1.0000x reference)
"""Trainium2 Bass kernel for nn_DCWTv2InferenceCache (segment-tree cached attention).

Sharding: tensor-parallel over the 16-head axis -> 8 cores x 2 heads.
Each core streams its (50000, 2*64) f32 slice of the value cache from HBM,
reduces segment-tree nodes to (64, 128) block-sums on the PE (selection-matrix
matmul accumulating in PSUM), then runs the per-node depth-projected attention
epilogue fully on-device. Output is head-sharded (2, 64) per core, gathered on
host. No cross-device communication.

v2 stream layout (vs the 127us whole-tile baseline):
  - the 25.2MB token stream goes down the Sync HWDGE ring as 14 sub-DMAs of
    ~1-2.1MB into separate tiles, so PE matmuls chase the DMA at 8-16-matmul
    granularity instead of waiting for a whole 8.4MB tile
  - constants go down the Scalar HWDGE ring and raw-token prefetch down the
    gpsimd SWDGE, so the stream's first descriptor issues right after the
    preamble instead of queueing behind 12 prefetch issues
  - the (128, 4096) r64 selection matrix is built on-device with two vector
    memsets (zero fill + stride-65 ones) instead of a 2.1MB DMA
  - epilogues are phase-split and injected into DMA-gated PE FIFO gaps; the
    mean scaling is folded into the softmax scale + Z normalization; the
    output accumulator is transposed (128, 2) so ONE output DMA suffices
"""

import math
import sys

if "/opt/trn_rl_repo" not in sys.path:
    sys.path.insert(0, "/opt/trn_rl_repo")

import numpy as np

import concourse.bass as bass
import concourse.mybir as mybir
import concourse.tile as tile
from concourse import bacc
from concourse.bass_utils import run_bass_kernel_spmd

# --- problem constants (from the reference nn.Module) ---
MAX_LEN = 65536
NUM_HEADS = 16
HEAD_DIM = 64
K_MAX = 64
LOCAL_WINDOW = 512
LOG_N = 17
LEAF_START = 2**LOG_N

N_CORES = 8
HPC = NUM_HEADS // N_CORES        # heads per core = 2
F = HPC * HEAD_DIM                # feature width per core = 128
NTOK = 50000                      # v_tokens buffer length

CHUNK = 128                       # tokens per matmul tile (partition dim)
BLK = CHUNK * K_MAX               # 8192 tokens per linear c-chunk (r64 path)
SEL64_W = K_MAX * 65              # 4160: 64 blocks of 64, viewed (64, 65)

f32 = mybir.dt.float32
f32r = mybir.dt.float32r
AF = mybir.ActivationFunctionType
AX = mybir.AxisListType

_last_results = None  # stash for test harness introspection


def _cblob_layout(NT):
    """Column offsets inside the packed (128, W) f32 constants blob."""
    nt = max(NT, 1)
    off = {}
    off["ident"] = 0
    off["qbd"] = 128
    off["qT"] = 130
    off["temps"] = 132
    off["msc"] = 132 + nt          # per-node mean_scale (folded into softmax)
    off["zsc"] = 132 + 2 * nt      # per-node NT/mean_scale (folded into Z)
    off["wTI"] = 132 + 3 * nt
    return off, 132 + 3 * nt + nt * 64


def cover_set(pos):
    """O(log n) segment-tree nodes covering prefix [0..pos-1]: (start, L, depth),
    ascending start / descending L (binary decomposition of pos)."""
    if pos <= 0:
        return []
    l, r = LEAF_START, LEAF_START + min(pos, MAX_LEN)
    out = []
    while l < r:
        if l & 1:
            d = LOG_N - int(math.floor(math.log2(l)))
            out.append(((l << d) - LEAF_START, 1 << d, d))
            l += 1
        if r & 1:
            r -= 1
            d = LOG_N - int(math.floor(math.log2(r)))
            out.append(((r << d) - LEAF_START, 1 << d, d))
        l >>= 1
        r >>= 1
    return sorted(out)


def _split_nodes(pos):
    nodes = cover_set(pos)
    big = [(s, L, d) for (s, L, d) in nodes if L > K_MAX]
    small = [(s, L, d) for (s, L, d) in nodes if L <= K_MAX]
    r64 = [(s, L, d) for (s, L, d) in big if L >= 2 * BLK and L % (2 * BLK) == 0]
    old = [(s, L, d) for (s, L, d) in big if not (L >= 2 * BLK and L % (2 * BLK) == 0)]
    return big, small, r64, old


def _build_program(pos):
    """Build the single-core Bass/Tile program (same program for all 8 cores)."""
    big, small, r64, old = _split_nodes(pos)
    # tree index order must match the host cblob: big (cover order) then small
    NT = len(big) + len(small)
    n_loc = min(pos, LOCAL_WINDOW)
    assert n_loc % CHUNK == 0, "local window must be chunk-aligned for this build"
    NLC = n_loc // CHUNK
    nt1 = max(NT, 1)

    inv_sqrt_d = 1.0 / math.sqrt(HEAD_DIM)

    nc = bacc.Bacc("TRN2", target_bir_lowering=False, debug=False)

    v = nc.dram_tensor("v", [NTOK, F], f32, kind="ExternalInput")
    selb_d = nc.dram_tensor("selb", [CHUNK, K_MAX], f32r, kind="ExternalInput")
    sel64row_d = nc.dram_tensor(
        "sel64row", [1, K_MAX * K_MAX], f32r, kind="ExternalInput"
    )
    CBOFF, CB_W = _cblob_layout(NT)
    cblob_d = nc.dram_tensor("cblob", [CHUNK, CB_W], f32, kind="ExternalInput")
    o = nc.dram_tensor("o", [F, HPC], f32, kind="ExternalOutput")

    with tile.TileContext(nc) as tc:
        with (
            tc.tile_pool(name="consts", bufs=1) as cpool,
            tc.tile_pool(name="vstream", bufs=5) as vpool,
            tc.tile_pool(name="fsb", bufs=3) as fpool,
            tc.tile_pool(name="ep_sb", bufs=2) as spool,
            tc.tile_pool(name="xsb", bufs=2) as xpool,
            tc.tile_pool(name="acc_ps", bufs=1, space=bass.MemorySpace.PSUM) as apool,
            tc.tile_pool(name="ep_ps", bufs=1, space=bass.MemorySpace.PSUM) as eppool,
            tc.tile_pool(name="out_ps", bufs=1, space=bass.MemorySpace.PSUM) as opool,
        ):
            # ---- constants on the Scalar HWDGE ring (2 issues only, so the
            # scalar compute queue is free early for the scale/qd chains);
            # raw-token prefetch on the gpsimd SWDGE; the big token stream owns
            # the Sync HWDGE ring so its first descriptor issues immediately.
            cb = cpool.tile([CHUNK, CB_W], f32)
            nc.scalar.dma_start(cb[:], cblob_d[:])
            ident_sb = cb[:, CBOFF["ident"] : CBOFF["ident"] + CHUNK]
            qbd_sb = cb[:, CBOFF["qbd"] : CBOFF["qbd"] + HPC]
            qT_sb = cb[0:HEAD_DIM, CBOFF["qT"] : CBOFF["qT"] + HPC]
            temps2_sb = cb[0:HPC, CBOFF["temps"] : CBOFF["temps"] + nt1]
            msc_sb = cb[0:HPC, CBOFF["msc"] : CBOFF["msc"] + nt1]
            zsc_sb = cb[0:HPC, CBOFF["zsc"] : CBOFF["zsc"] + nt1]

            def wTI_slice(n):
                c0 = CBOFF["wTI"] + n * HEAD_DIM
                return cb[0:HEAD_DIM, c0 : c0 + HEAD_DIM]

            # eye fold matrix for the old path (tokens mod 64 within a chunk)
            sel_t = cpool.tile([CHUNK, K_MAX], f32r)
            nc.scalar.dma_start(sel_t[:], selb_d[:])
            sel_sb = sel_t[:]

            # small tail nodes + local window + old-path chunks via SWDGE
            small_tiles = []
            for si, (start_s, L_s, _d) in enumerate(small):
                fsm = cpool.tile([K_MAX, F], f32, name=f"fsm{si}", tag=f"fsm{si}")
                nc.gpsimd.dma_start(fsm[0:L_s, :], v[start_s : start_s + L_s, :])
                small_tiles.append(fsm)
            lstart = pos - n_loc
            fl_sb = cpool.tile([CHUNK, NLC, F], f32)
            nc.gpsimd.dma_start(
                fl_sb[:],
                v[lstart : lstart + n_loc, :].rearrange("(c p) f -> p c f", p=CHUNK),
            )
            oldpath_tiles = {}
            for (start_b, L_b, _d) in old:
                nch_b = L_b // CHUNK
                vo = cpool.tile(
                    [CHUNK, nch_b, F], f32r, name=f"vo{start_b}", tag=f"vo{start_b}"
                )
                for cb_i in range(nch_b):
                    srcb = v[start_b + cb_i * CHUNK : start_b + (cb_i + 1) * CHUNK, :]
                    nc.gpsimd.dma_start(vo[:, cb_i, :], srcb.bitcast(f32r))
                oldpath_tiles[start_b] = vo

            # ---- r64 selection matrix: lhsT block r must have column r
            # all-ones, i.e. flat position r*64 + r = r*65 hot; identical on
            # every partition. The DRAM image is a single 16KB row DMA'd with
            # a partition-broadcast source AP (engines cannot tag f32r, DMAs
            # can), so it costs ~16KB of HBM instead of 2.1MB.
            sel64 = cpool.tile([CHUNK, K_MAX * K_MAX], f32r)
            nc.scalar.dma_start(
                sel64[:], sel64row_d[:].broadcast_to([CHUNK, K_MAX * K_MAX])
            )

            def sel64_lhsT(r):
                return sel64[:, r * K_MAX : (r + 1) * K_MAX]

            # ---- per-node softmax scales: msc/((softplus(t)+1e-6)*sqrt(D)) ----
            et_sb = cpool.tile([HPC, nt1], f32)
            nc.scalar.activation(et_sb[:], temps2_sb, AF.Exp)
            sp_sb = cpool.tile([HPC, nt1], f32)
            nc.scalar.activation(sp_sb[:], et_sb[:], AF.Ln, bias=1.0)  # softplus
            u_sb = cpool.tile([HPC, nt1], f32)
            nc.scalar.mul(u_sb[:], sp_sb, math.sqrt(HEAD_DIM))
            nc.vector.tensor_scalar_add(u_sb[:], u_sb[:], 1e-6 * math.sqrt(HEAD_DIM))
            r0_sb = cpool.tile([HPC, nt1], f32)
            nc.vector.reciprocal(r0_sb[:], u_sb[:])
            rs_sb = cpool.tile([HPC, nt1], f32)
            nc.vector.tensor_mul(rs_sb[:], r0_sb[:], msc_sb)   # fold mean scale
            ns_sb = cpool.tile([HPC, nt1], f32)
            nc.scalar.mul(ns_sb[:], rs_sb[:], -1.0)

            # ---- all tree-node q_depth projections upfront (block-diag);
            # PE runs these while the first stream sub-DMA is in flight.
            qd_all = cpool.tile([2 * HEAD_DIM, nt1, HPC], f32)
            nc.vector.memset(qd_all[:], 0.0)
            for n in range(NT):
                qd_ps = eppool.tile([2 * HEAD_DIM, HPC], f32, tag="qd_ps")
                nc.tensor.matmul(
                    qd_ps[0:HEAD_DIM, 0:1],
                    wTI_slice(n), qT_sb[:, 0:1], start=True, stop=True,
                )
                nc.tensor.matmul(
                    qd_ps[HEAD_DIM : 2 * HEAD_DIM, 1:2],
                    wTI_slice(n), qT_sb[:, 1:2], start=True, stop=True,
                )
                nc.scalar.copy(qd_all[0:HEAD_DIM, n, 0:1], qd_ps[0:HEAD_DIM, 0:1])
                nc.scalar.copy(
                    qd_all[HEAD_DIM : 2 * HEAD_DIM, n, 1:2],
                    qd_ps[HEAD_DIM : 2 * HEAD_DIM, 1:2],
                )

            # ---- cross-node TRANSPOSED output accumulator (128, 2) PSUM ----
            out_ps = opool.tile([F, HPC], f32)
            n_out_mm = NT + NLC
            out_mm = [0]

            def out_matmul(f_sb_ap, wT_sb_ap):
                # out_T[(h d), h'] += sum_k f[k, (h d)] * wT[k, h']
                nc.tensor.matmul(
                    out_ps[:], f_sb_ap, wT_sb_ap,
                    start=(out_mm[0] == 0), stop=(out_mm[0] == n_out_mm - 1),
                )
                out_mm[0] += 1

            def softmax_weights(s_ps_ap, K, node_i, is_tree, wtag="wsb", wbufs=6):
                """softmax over K free-dim entries of (2, K) logits (pre-scale);
                tree nodes fold mean_scale/NT into the weights via zsc."""
                smax = xpool.tile([HPC, 1], f32, tag="smax" + wtag, bufs=wbufs)
                nc.vector.reduce_max(smax[:], s_ps_ap, axis=AX.X)
                biast = xpool.tile([HPC, 1], f32, tag="biast" + wtag, bufs=wbufs)
                ebd = xpool.tile([HPC, K], f32, tag="esb" + wtag, bufs=wbufs)
                zt = xpool.tile([HPC, 1], f32, tag="zt" + wtag, bufs=wbufs)
                if is_tree:
                    nc.vector.tensor_scalar_mul(
                        biast[:], smax[:], ns_sb[:, node_i : node_i + 1]
                    )
                    nc.scalar.activation(
                        ebd[:], s_ps_ap, AF.Exp,
                        bias=biast[:], scale=rs_sb[:, node_i : node_i + 1],
                        accum_out=zt[:],
                    )
                    zs = xpool.tile([HPC, 1], f32, tag="zs" + wtag, bufs=wbufs)
                    nc.vector.tensor_scalar_mul(
                        zs[:], zt[:], zsc_sb[:, node_i : node_i + 1]
                    )
                    zt = zs
                else:
                    nc.scalar.mul(biast[:], smax[:], -inv_sqrt_d)
                    nc.scalar.activation(
                        ebd[:], s_ps_ap, AF.Exp, bias=biast[:], scale=inv_sqrt_d,
                        accum_out=zt[:],
                    )
                rz = xpool.tile([HPC, 1], f32, tag="rz" + wtag, bufs=wbufs)
                nc.vector.reciprocal(rz[:], zt[:])
                w_sb = xpool.tile([HPC, K], f32, tag=wtag, bufs=wbufs)
                nc.vector.tensor_scalar_mul(w_sb[:], ebd[:], rz[:])
                return w_sb

            # ---- phase-split tree epilogue. PSUM tags are shared with the
            # local-window epilogue (padded allocs) to stay within 8 banks:
            # acc(1) acco(1) qd(1) fT(1) s(2) wT(1) out(1) = 8.
            def tree_phase1(node_i, f_sb_ap, K):
                fT_ps = eppool.tile([F, CHUNK], f32, tag="fT_ps")
                nc.tensor.transpose(fT_ps[:, 0:K], f_sb_ap, ident_sb[0:K, 0:K])
                fT_sb = spool.tile([F, K_MAX], f32, tag="fT_sb")
                nc.scalar.copy(fT_sb[:, 0:K], fT_ps[:, 0:K])
                s_ps = eppool.tile([HPC, NLC * CHUNK], f32, tag="s_ps", bufs=2)
                nc.tensor.matmul(
                    s_ps[:, 0:K], qd_all[:, node_i, :], fT_sb[:, 0:K],
                    start=True, stop=True,
                )
                return softmax_weights(s_ps[:, 0:K], K, node_i, True)

            def tree_phase2(f_sb_ap, w_sb, K):
                wT_ps = eppool.tile([CHUNK, HPC], f32, tag="wT_ps")
                nc.tensor.transpose(wT_ps[0:K, :], w_sb[:], ident_sb[0:HPC, 0:HPC])
                wT_sb = spool.tile([K_MAX, HPC], f32, tag="wT_sb")
                nc.scalar.copy(wT_sb[0:K, :], wT_ps[0:K, :])
                out_matmul(f_sb_ap, wT_sb[0:K, :])

            # ---- local window epilogue, phase-split the same way ----
            local_state = {}

            def local_phase1():
                fTl_sb = spool.tile([F, NLC * CHUNK], f32, tag="fTl_sb", bufs=1)
                for c in range(NLC):
                    fTl_ps = eppool.tile([F, CHUNK], f32, tag="fT_ps")
                    nc.tensor.transpose(fTl_ps[:], fl_sb[:, c, :], ident_sb[:])
                    nc.scalar.copy(
                        fTl_sb[:, c * CHUNK : (c + 1) * CHUNK], fTl_ps[:]
                    )
                sl_ps = eppool.tile([HPC, NLC * CHUNK], f32, tag="s_ps", bufs=2)
                nc.tensor.matmul(sl_ps[:], qbd_sb, fTl_sb[:], start=True, stop=True)
                local_state["w"] = softmax_weights(
                    sl_ps[:], n_loc, -1, False, wtag="wlsb", wbufs=1
                )

            def local_phase2():
                wl_sb = local_state["w"]
                for c in range(NLC):
                    wTl_ps = eppool.tile([CHUNK, HPC], f32, tag="wT_ps")
                    nc.tensor.transpose(
                        wTl_ps[:], wl_sb[:, c * CHUNK : (c + 1) * CHUNK],
                        ident_sb[0:HPC, 0:HPC],
                    )
                    wTl_sb = spool.tile([CHUNK, HPC], f32, tag="wTl_sb")
                    nc.scalar.copy(wTl_sb[:], wTl_ps[:])
                    out_matmul(fl_sb[:, c, :], wTl_sb[:])

            # ---- old-path (sub-BLK) node block sums ----
            def emit_old_mms(start, L):
                nch = L // CHUNK
                vt = oldpath_tiles[start]
                ps2 = apool.tile([K_MAX, 2, F], f32, tag="acco")
                done = 0
                c = 0
                while c < nch:
                    w = 2 if c + 2 <= nch else 1
                    nc.tensor.matmul(
                        ps2[:, 0:w, :], sel_sb, vt[:, c : c + w, :],
                        start=(done == 0), stop=(done + w == nch),
                    )
                    done += w
                    c += w
                f_sb = fpool.tile([K_MAX, F], f32, tag="f")
                if nch > 1:
                    nc.vector.tensor_reduce(
                        f_sb[:], ps2[:, :, :].rearrange("p c f -> p f c"),
                        axis=AX.X, op=mybir.AluOpType.add,
                    )
                else:
                    nc.scalar.copy(f_sb[:], ps2[:, 0, :])
                return f_sb

            # ---- r64 stream: sub-DMAs (r-ranges) into separate tiles ----
            stream_acc = {}   # node_i -> psum tile

            def emit_iteration(node_i, start, L, c0, subs, fillers):
                """One 2-BLK iteration of r64 node `node_i`: len(subs) sub-DMAs
                on the sync ring, then per-sub matmul groups. fillers[-1] runs
                after the DMA issues (before any matmul); fillers[j] after sub
                j's matmul group — epilogue work placed into DMA-gated gaps."""
                CC = L // BLK
                n_mm_node = ((CC + 1) // 2) * K_MAX
                base = v[start + c0 * BLK : start + (c0 + 2) * BLK, :]
                src4 = base.bitcast(f32r).rearrange(
                    "(c q r) f -> q c r f", q=CHUNK, r=K_MAX
                )
                tiles = []
                for (a, b) in subs:
                    w = b - a
                    vt = vpool.tile(
                        [CHUNK, 2, w, F], f32r, tag=f"vs{w}",
                        bufs=(5 if w >= 16 else 4),
                    )
                    nc.sync.dma_start(vt[:], src4[:, :, a:b, :])
                    tiles.append(vt)
                for fn in fillers.get(-1, []):
                    fn()
                if node_i not in stream_acc:
                    stream_acc[node_i] = apool.tile(
                        [K_MAX, 2, F], f32, tag="acc", name=f"acc{node_i}"
                    )
                ps2 = stream_acc[node_i]
                done = (c0 // 2) * K_MAX
                for j, (a, b) in enumerate(subs):
                    vt = tiles[j]
                    for r in range(a, b):
                        nc.tensor.matmul(
                            ps2[:, :, :], sel64_lhsT(r), vt[:, :, r - a, :],
                            start=(done == 0), stop=(done == n_mm_node - 1),
                        )
                        done += 1
                    for fn in fillers.get(j, []):
                        fn()

            def stream_f_add(node_i):
                ps2 = stream_acc[node_i]
                f_sb = fpool.tile([K_MAX, F], f32, tag="f")
                nc.vector.tensor_reduce(
                    f_sb[:], ps2[:, :, :].rearrange("p c f -> p f c"),
                    axis=AX.X, op=mybir.AluOpType.add,
                )
                return f_sb

            # ================= emission schedule =================
            SUBS4 = [(0, 16), (16, 32), (32, 48), (48, 64)]
            SUBS6 = [(0, 16), (16, 32), (32, 40), (40, 48), (48, 56), (56, 64)]

            iters = []   # (r64_node_idx, start, L, c0, node_last)
            for ni, (s_, L_, _d) in enumerate(r64):
                CC = L_ // BLK
                for c0 in range(0, CC, 2):
                    iters.append((ni, s_, L_, c0, c0 + 2 >= CC))
            NIT = len(iters)

            st = {}

            def fill_local1():
                local_phase1()

            def fill_smalls1():
                for si, (s_, L_, _d) in enumerate(small):
                    st[f"wsm{si}"] = tree_phase1(
                        len(big) + si, small_tiles[si][0:L_, :], L_
                    )

            def fill_old_mms():
                for oi, (s_, L_, _d) in enumerate(old):
                    st[f"fo{oi}"] = emit_old_mms(s_, L_)

            def fill_old1():
                for oi, (s_, L_, _d) in enumerate(old):
                    st[f"wo{oi}"] = tree_phase1(len(r64) + oi, st[f"fo{oi}"], K_MAX)

            def fill_local2():
                local_phase2()

            def fill_smalls2():
                for si, (s_, L_, _d) in enumerate(small):
                    tree_phase2(small_tiles[si][0:L_, :], st[f"wsm{si}"], L_)

            def fill_old2():
                for oi, (s_, L_, _d) in enumerate(old):
                    tree_phase2(st[f"fo{oi}"], st[f"wo{oi}"], K_MAX)

            def fill_r64_f_add(ni):
                def f():
                    st[f"fr{ni}"] = stream_f_add(ni)
                return f

            def fill_r64_1(ni):
                def f():
                    st[f"wr{ni}"] = tree_phase1(ni, st[f"fr{ni}"], K_MAX)
                return f

            def fill_r64_2(ni):
                def f():
                    tree_phase2(st[f"fr{ni}"], st[f"wr{ni}"], K_MAX)
                return f

            filler_maps = [dict() for _ in range(max(NIT, 1))]

            def add_fill(it, sub, fn):
                filler_maps[it].setdefault(sub, []).append(fn)

            if NIT >= 3:
                add_fill(0, 0, fill_local1)
                add_fill(0, 1, fill_smalls1)
                add_fill(0, 2, fill_old_mms)
                add_fill(0, 3, fill_old1)
                add_fill(1, 0, fill_local2)
                add_fill(1, 1, fill_smalls2)
                add_fill(1, 2, fill_old2)
            else:
                for fn in [fill_local1, fill_smalls1, fill_old_mms, fill_old1,
                           fill_local2, fill_smalls2, fill_old2]:
                    add_fill(0, 0, fn)

            # r64 node boundary epilogues: f_add + phase1 emitted BEFORE the
            # next node's first matmul (PSUM slot WAR), phase2 two sub-groups
            # later so its softmax is done; the final node's epilogue is the
            # tail.
            for it_i, (ni, s_, L_, c0, node_last) in enumerate(iters):
                if not node_last or it_i + 1 >= NIT:
                    continue
                add_fill(it_i + 1, -1, fill_r64_f_add(ni))
                add_fill(it_i + 1, -1, fill_r64_1(ni))
                add_fill(it_i + 1, 2, fill_r64_2(ni))

            for it_i, (ni, s_, L_, c0, node_last) in enumerate(iters):
                subs = SUBS6 if it_i == NIT - 1 else SUBS4
                emit_iteration(ni, s_, L_, c0, subs, filler_maps[it_i])

            if NIT:
                ni_last = iters[-1][0]
                fill_r64_f_add(ni_last)()
                fill_r64_1(ni_last)()
                fill_r64_2(ni_last)()

            # ================= final output =================
            acc_sb = spool.tile([F, HPC], f32, tag="acc_sb")
            nc.scalar.copy(acc_sb[:], out_ps[:])
            nc.sync.dma_start(o[:], acc_sb[:])

    nc.compile()
    return nc


def _make_in_maps(v_tokens, q_new, depth_proj_w, depth_temp, pos):
    big, small, r64, old = _split_nodes(pos)
    tree = big + small
    NT = len(tree)
    OFF, CB_W = _cblob_layout(NT)
    nt1 = max(NT, 1)

    sel = np.tile(np.eye(K_MAX, dtype=np.float32), (CHUNK // K_MAX, 1))
    sel64row = np.zeros((1, K_MAX * K_MAX), np.float32)
    sel64row[0, ::65] = 1.0

    wTI = np.stack(
        [np.eye(HEAD_DIM, dtype=np.float32) + depth_proj_w[d].T for (_, _, d) in tree]
    ) if NT else np.zeros((1, HEAD_DIM, HEAD_DIM), np.float32)
    tsel = np.array([depth_temp[d] for (_, _, d) in tree], np.float32) \
        if NT else np.zeros((1,), np.float32)
    msc = np.array(
        [float(K_MAX) / L if L > K_MAX else 1.0 for (_, L, _d) in tree], np.float32
    ) if NT else np.ones((1,), np.float32)
    zsc = (float(NT) / msc).astype(np.float32) if NT else np.ones((1,), np.float32)

    in_maps = []
    for c in range(N_CORES):
        q_c = q_new[0, HPC * c : HPC * (c + 1), :]          # (2, 64)
        cb = np.zeros((CHUNK, CB_W), np.float32)
        cb[:, OFF["ident"] : OFF["ident"] + CHUNK] = np.eye(CHUNK)
        for h in range(HPC):
            cb[h * HEAD_DIM : (h + 1) * HEAD_DIM, OFF["qbd"] + h] = q_c[h]
        cb[0:HEAD_DIM, OFF["qT"] : OFF["qT"] + HPC] = q_c.T
        cb[0:HPC, OFF["temps"] : OFF["temps"] + nt1] = tsel[None, :]
        cb[0:HPC, OFF["msc"] : OFF["msc"] + nt1] = msc[None, :]
        cb[0:HPC, OFF["zsc"] : OFF["zsc"] + nt1] = zsc[None, :]
        for n in range(nt1):
            cb[0:HEAD_DIM, OFF["wTI"] + n * HEAD_DIM : OFF["wTI"] + (n + 1) * HEAD_DIM] = (
                wTI[n] if NT else 0.0
            )
        im = {
            "v": np.ascontiguousarray(
                v_tokens[:, HPC * c : HPC * (c + 1), :]
            ).reshape(NTOK, F),
            "selb": np.ascontiguousarray(sel),
            "sel64row": sel64row,
            "cblob": cb,
        }
        in_maps.append(im)
    return in_maps


def kernel(v_tokens, q_new, depth_proj_w, depth_temp, n_tokens, _profile=False):
    global _last_results
    v_tokens = np.asarray(v_tokens, dtype=np.float32)
    q_new = np.asarray(q_new, dtype=np.float32)
    depth_proj_w = np.asarray(depth_proj_w, dtype=np.float32)
    depth_temp = np.asarray(depth_temp, dtype=np.float32)
    pos = int(n_tokens)

    nc = _build_program(pos)
    in_maps = _make_in_maps(v_tokens, q_new, depth_proj_w, depth_temp, pos)
    res = run_bass_kernel_spmd(
        nc, in_maps, core_ids=list(range(N_CORES)), trace=_profile
    )
    _last_results = res

    out = np.zeros((1, NUM_HEADS, HEAD_DIM), np.float32)
    for c in range(N_CORES):
        oT = res.results[c]["o"]                      # (128, 2)
        for h in range(HPC):
            out[0, HPC * c + h, :] = oT[h * HEAD_DIM : (h + 1) * HEAD_DIM, h]
    return out


# revision 20
# speedup vs baseline: 1.3876x; 1.3876x over previous
"""Trainium2 Bass kernel for nn_DCWTv2InferenceCache (segment-tree cached attention).

Sharding: tensor-parallel over the 16-head axis -> 8 cores x 2 heads.
Each core streams its (50000, 2*64) f32 slice of the value cache from HBM,
reduces segment-tree nodes to (64, 128) block-sums on the PE (selection-matrix
matmul accumulating in PSUM), then runs the per-node depth-projected attention
epilogue fully on-device. Output is head-sharded (2, 64) per core, gathered on
host. No cross-device communication.

v2 stream layout (vs the 127us whole-tile baseline):
  - the 25.2MB token stream goes down the Sync HWDGE ring as 14 sub-DMAs of
    ~1-2.1MB into separate tiles, so PE matmuls chase the DMA at 8-16-matmul
    granularity instead of waiting for a whole 8.4MB tile
  - constants go down the Scalar HWDGE ring and raw-token prefetch down the
    gpsimd SWDGE, so the stream's first descriptor issues right after the
    preamble instead of queueing behind 12 prefetch issues
  - the (128, 4096) r64 selection matrix is built on-device with two vector
    memsets (zero fill + stride-65 ones) instead of a 2.1MB DMA
  - epilogues are phase-split and injected into DMA-gated PE FIFO gaps; the
    mean scaling is folded into the softmax scale + Z normalization; the
    output accumulator is transposed (128, 2) so ONE output DMA suffices
"""

import math
import sys

if "/opt/trn_rl_repo" not in sys.path:
    sys.path.insert(0, "/opt/trn_rl_repo")

import numpy as np

import concourse.bass as bass
import concourse.mybir as mybir
import concourse.tile as tile
from concourse import bacc
from concourse.bass_utils import run_bass_kernel_spmd

# --- problem constants (from the reference nn.Module) ---
MAX_LEN = 65536
NUM_HEADS = 16
HEAD_DIM = 64
K_MAX = 64
LOCAL_WINDOW = 512
LOG_N = 17
LEAF_START = 2**LOG_N

N_CORES = 8
HPC = NUM_HEADS // N_CORES        # heads per core = 2
F = HPC * HEAD_DIM                # feature width per core = 128
NTOK = 50000                      # v_tokens buffer length

CHUNK = 128                       # tokens per matmul tile (partition dim)
BLK = CHUNK * K_MAX               # 8192 tokens per linear c-chunk (r64 path)
SEL64_W = K_MAX * 65              # 4160: 64 blocks of 64, viewed (64, 65)

f32 = mybir.dt.float32
f32r = mybir.dt.float32r
AF = mybir.ActivationFunctionType
AX = mybir.AxisListType

_last_results = None  # stash for test harness introspection


def _cblob_layout(NT):
    """Column offsets inside the packed (128, W) f32 constants blob."""
    nt = max(NT, 1)
    off = {}
    off["ident"] = 0
    off["qbd"] = 128
    off["qT"] = 130
    off["temps"] = 132
    off["msc"] = 132 + nt          # per-node mean_scale (folded into softmax)
    off["zsc"] = 132 + 2 * nt      # per-node NT/mean_scale (folded into Z)
    off["wTI"] = 132 + 3 * nt
    return off, 132 + 3 * nt + nt * 64


def cover_set(pos):
    """O(log n) segment-tree nodes covering prefix [0..pos-1]: (start, L, depth),
    ascending start / descending L (binary decomposition of pos)."""
    if pos <= 0:
        return []
    l, r = LEAF_START, LEAF_START + min(pos, MAX_LEN)
    out = []
    while l < r:
        if l & 1:
            d = LOG_N - int(math.floor(math.log2(l)))
            out.append(((l << d) - LEAF_START, 1 << d, d))
            l += 1
        if r & 1:
            r -= 1
            d = LOG_N - int(math.floor(math.log2(r)))
            out.append(((r << d) - LEAF_START, 1 << d, d))
        l >>= 1
        r >>= 1
    return sorted(out)


def _split_nodes(pos):
    nodes = cover_set(pos)
    big = [(s, L, d) for (s, L, d) in nodes if L > K_MAX]
    small = [(s, L, d) for (s, L, d) in nodes if L <= K_MAX]
    r64 = [(s, L, d) for (s, L, d) in big if L >= 2 * BLK and L % (2 * BLK) == 0]
    old = [(s, L, d) for (s, L, d) in big if not (L >= 2 * BLK and L % (2 * BLK) == 0)]
    return big, small, r64, old


def _build_program(pos):
    """Build the single-core Bass/Tile program (same program for all 8 cores)."""
    big, small, r64, old = _split_nodes(pos)
    # tree index order must match the host cblob: big (cover order) then small
    NT = len(big) + len(small)
    n_loc = min(pos, LOCAL_WINDOW)
    assert n_loc % CHUNK == 0, "local window must be chunk-aligned for this build"
    NLC = n_loc // CHUNK
    nt1 = max(NT, 1)

    inv_sqrt_d = 1.0 / math.sqrt(HEAD_DIM)

    nc = bacc.Bacc("TRN2", target_bir_lowering=False, debug=False)

    v = nc.dram_tensor("v", [NTOK, F], f32, kind="ExternalInput")
    selb_d = nc.dram_tensor("selb", [CHUNK, K_MAX], f32r, kind="ExternalInput")
    sel64_d = nc.dram_tensor(
        "sel64", [4, CHUNK, 16 * K_MAX], f32r, kind="ExternalInput"
    )
    CBOFF, CB_W = _cblob_layout(NT)
    cblob_d = nc.dram_tensor("cblob", [CHUNK, CB_W], f32, kind="ExternalInput")
    o = nc.dram_tensor("o", [F, HPC], f32, kind="ExternalOutput")

    with tile.TileContext(nc) as tc:
        with (
            tc.tile_pool(name="consts", bufs=1) as cpool,
            tc.tile_pool(name="vstream", bufs=5) as vpool,
            tc.tile_pool(name="fsb", bufs=3) as fpool,
            tc.tile_pool(name="ep_sb", bufs=2) as spool,
            tc.tile_pool(name="xsb", bufs=2) as xpool,
            tc.tile_pool(name="acc_ps", bufs=1, space=bass.MemorySpace.PSUM) as apool,
            tc.tile_pool(name="ep_ps", bufs=1, space=bass.MemorySpace.PSUM) as eppool,
            tc.tile_pool(name="out_ps", bufs=1, space=bass.MemorySpace.PSUM) as opool,
        ):
            # ---- constants on the Scalar HWDGE ring (2 issues only, so the
            # scalar compute queue is free early for the scale/qd chains);
            # raw-token prefetch on the gpsimd SWDGE; the big token stream owns
            # the Sync HWDGE ring so its first descriptor issues immediately.
            cb = cpool.tile([CHUNK, CB_W], f32)
            nc.scalar.dma_start(cb[:], cblob_d[:])
            ident_sb = cb[:, CBOFF["ident"] : CBOFF["ident"] + CHUNK]
            qbd_sb = cb[:, CBOFF["qbd"] : CBOFF["qbd"] + HPC]
            qT_sb = cb[0:HEAD_DIM, CBOFF["qT"] : CBOFF["qT"] + HPC]
            temps2_sb = cb[0:HPC, CBOFF["temps"] : CBOFF["temps"] + nt1]
            msc_sb = cb[0:HPC, CBOFF["msc"] : CBOFF["msc"] + nt1]
            zsc_sb = cb[0:HPC, CBOFF["zsc"] : CBOFF["zsc"] + nt1]

            def wTI_slice(n):
                c0 = CBOFF["wTI"] + n * HEAD_DIM
                return cb[0:HEAD_DIM, c0 : c0 + HEAD_DIM]

            # ---- r64 selection matrix, 4 part-tiles of 16 r-blocks each so
            # the first stream matmuls are gated only by part 0 (0.52MB).
            # Scalar-ring order below is by first-use time.
            sel64_t = []
            for j in range(4):
                s64 = cpool.tile(
                    [CHUNK, 16 * K_MAX], f32r, name=f"sel64_{j}", tag=f"sel64_{j}"
                )
                sel64_t.append(s64)
            nc.scalar.dma_start(sel64_t[0][:], sel64_d[0])
            nc.scalar.dma_start(sel64_t[1][:], sel64_d[1])

            def sel64_lhsT(r):
                j, k = r // 16, r % 16
                return sel64_t[j][:, k * K_MAX : (k + 1) * K_MAX]

            # eye fold matrix for the old path (tokens mod 64 within a chunk)
            sel_t = cpool.tile([CHUNK, K_MAX], f32r)
            nc.scalar.dma_start(sel_t[:], selb_d[:])
            sel_sb = sel_t[:]

            # old-path nodes (one DMA each) + small tail nodes + local window
            oldpath_tiles = {}
            for (start_b, L_b, _d) in old:
                nch_b = L_b // CHUNK
                vo = cpool.tile(
                    [CHUNK, nch_b, F], f32r, name=f"vo{start_b}", tag=f"vo{start_b}"
                )
                src_o = v[start_b : start_b + L_b, :].bitcast(f32r)
                nc.scalar.dma_start(
                    vo[:], src_o.rearrange("(c p) f -> p c f", p=CHUNK)
                )
                oldpath_tiles[start_b] = vo
            small_tiles = []
            for si, (start_s, L_s, _d) in enumerate(small):
                fsm = cpool.tile([K_MAX, F], f32, name=f"fsm{si}", tag=f"fsm{si}")
                nc.scalar.dma_start(fsm[0:L_s, :], v[start_s : start_s + L_s, :])
                small_tiles.append(fsm)
            lstart = pos - n_loc
            fl_sb = cpool.tile([CHUNK, NLC, F], f32)
            nc.scalar.dma_start(
                fl_sb[:],
                v[lstart : lstart + n_loc, :].rearrange("(c p) f -> p c f", p=CHUNK),
            )
            nc.scalar.dma_start(sel64_t[2][:], sel64_d[2])
            nc.scalar.dma_start(sel64_t[3][:], sel64_d[3])

            # ---- per-node softmax scales: msc/((softplus(t)+1e-6)*sqrt(D)) ----
            et_sb = cpool.tile([HPC, nt1], f32)
            nc.scalar.activation(et_sb[:], temps2_sb, AF.Exp)
            sp_sb = cpool.tile([HPC, nt1], f32)
            nc.scalar.activation(sp_sb[:], et_sb[:], AF.Ln, bias=1.0)  # softplus
            u_sb = cpool.tile([HPC, nt1], f32)
            nc.scalar.mul(u_sb[:], sp_sb, math.sqrt(HEAD_DIM))
            nc.vector.tensor_scalar_add(u_sb[:], u_sb[:], 1e-6 * math.sqrt(HEAD_DIM))
            r0_sb = cpool.tile([HPC, nt1], f32)
            nc.vector.reciprocal(r0_sb[:], u_sb[:])
            rs_sb = cpool.tile([HPC, nt1], f32)
            nc.vector.tensor_mul(rs_sb[:], r0_sb[:], msc_sb)   # fold mean scale
            ns_sb = cpool.tile([HPC, nt1], f32)
            nc.scalar.mul(ns_sb[:], rs_sb[:], -1.0)

            # ---- all tree-node q_depth projections upfront (block-diag);
            # PE runs these while the first stream sub-DMA is in flight.
            qd_all = cpool.tile([2 * HEAD_DIM, nt1, HPC], f32)
            nc.vector.memset(qd_all[:], 0.0)
            for n in range(NT):
                qd_ps = eppool.tile([2 * HEAD_DIM, HPC], f32, tag="qd_ps")
                nc.tensor.matmul(
                    qd_ps[0:HEAD_DIM, 0:1],
                    wTI_slice(n), qT_sb[:, 0:1], start=True, stop=True,
                )
                nc.tensor.matmul(
                    qd_ps[HEAD_DIM : 2 * HEAD_DIM, 1:2],
                    wTI_slice(n), qT_sb[:, 1:2], start=True, stop=True,
                )
                nc.scalar.copy(qd_all[0:HEAD_DIM, n, 0:1], qd_ps[0:HEAD_DIM, 0:1])
                nc.scalar.copy(
                    qd_all[HEAD_DIM : 2 * HEAD_DIM, n, 1:2],
                    qd_ps[HEAD_DIM : 2 * HEAD_DIM, 1:2],
                )

            # ---- cross-node TRANSPOSED output accumulator (128, 2) PSUM ----
            out_ps = opool.tile([F, HPC], f32)
            n_out_mm = NT + NLC
            out_mm = [0]

            def out_matmul(f_sb_ap, wT_sb_ap):
                # out_T[(h d), h'] += sum_k f[k, (h d)] * wT[k, h']
                nc.tensor.matmul(
                    out_ps[:], f_sb_ap, wT_sb_ap,
                    start=(out_mm[0] == 0), stop=(out_mm[0] == n_out_mm - 1),
                )
                out_mm[0] += 1

            def softmax_weights(s_ps_ap, K, node_i, is_tree, wtag="wsb", wbufs=6):
                """softmax over K free-dim entries of (2, K) logits (pre-scale);
                tree nodes fold mean_scale/NT into the weights via zsc."""
                smax = xpool.tile([HPC, 1], f32, tag="smax" + wtag, bufs=wbufs)
                nc.vector.reduce_max(smax[:], s_ps_ap, axis=AX.X)
                biast = xpool.tile([HPC, 1], f32, tag="biast" + wtag, bufs=wbufs)
                ebd = xpool.tile([HPC, K], f32, tag="esb" + wtag, bufs=wbufs)
                zt = xpool.tile([HPC, 1], f32, tag="zt" + wtag, bufs=wbufs)
                if is_tree:
                    nc.vector.tensor_scalar_mul(
                        biast[:], smax[:], ns_sb[:, node_i : node_i + 1]
                    )
                    nc.scalar.activation(
                        ebd[:], s_ps_ap, AF.Exp,
                        bias=biast[:], scale=rs_sb[:, node_i : node_i + 1],
                        accum_out=zt[:],
                    )
                    zs = xpool.tile([HPC, 1], f32, tag="zs" + wtag, bufs=wbufs)
                    nc.vector.tensor_scalar_mul(
                        zs[:], zt[:], zsc_sb[:, node_i : node_i + 1]
                    )
                    zt = zs
                else:
                    nc.scalar.mul(biast[:], smax[:], -inv_sqrt_d)
                    nc.scalar.activation(
                        ebd[:], s_ps_ap, AF.Exp, bias=biast[:], scale=inv_sqrt_d,
                        accum_out=zt[:],
                    )
                rz = xpool.tile([HPC, 1], f32, tag="rz" + wtag, bufs=wbufs)
                nc.vector.reciprocal(rz[:], zt[:])
                w_sb = xpool.tile([HPC, K], f32, tag=wtag, bufs=wbufs)
                nc.vector.tensor_scalar_mul(w_sb[:], ebd[:], rz[:])
                return w_sb

            # ---- phase-split tree epilogue. PSUM tags are shared with the
            # local-window epilogue (padded allocs) to stay within 8 banks:
            # acc(1) acco(1) qd(1) fT(1) s(2) wT(1) out(1) = 8.
            def tree_phase1(node_i, f_sb_ap, K):
                fT_ps = eppool.tile([F, CHUNK], f32, tag="fT_ps")
                nc.tensor.transpose(fT_ps[:, 0:K], f_sb_ap, ident_sb[0:K, 0:K])
                fT_sb = spool.tile([F, K_MAX], f32, tag="fT_sb")
                nc.scalar.copy(fT_sb[:, 0:K], fT_ps[:, 0:K])
                s_ps = eppool.tile([HPC, NLC * CHUNK], f32, tag="s_ps", bufs=2)
                nc.tensor.matmul(
                    s_ps[:, 0:K], qd_all[:, node_i, :], fT_sb[:, 0:K],
                    start=True, stop=True,
                )
                return softmax_weights(s_ps[:, 0:K], K, node_i, True)

            def tree_phase2(f_sb_ap, w_sb, K):
                wT_ps = eppool.tile([CHUNK, HPC], f32, tag="wT_ps")
                nc.tensor.transpose(wT_ps[0:K, :], w_sb[:], ident_sb[0:HPC, 0:HPC])
                wT_sb = spool.tile([K_MAX, HPC], f32, tag="wT_sb")
                nc.scalar.copy(wT_sb[0:K, :], wT_ps[0:K, :])
                out_matmul(f_sb_ap, wT_sb[0:K, :])

            # ---- local window epilogue, phase-split the same way ----
            local_state = {}

            def local_phase1():
                fTl_sb = spool.tile([F, NLC * CHUNK], f32, tag="fTl_sb", bufs=1)
                for c in range(NLC):
                    fTl_ps = eppool.tile([F, CHUNK], f32, tag="fT_ps")
                    nc.tensor.transpose(fTl_ps[:], fl_sb[:, c, :], ident_sb[:])
                    nc.scalar.copy(
                        fTl_sb[:, c * CHUNK : (c + 1) * CHUNK], fTl_ps[:]
                    )
                sl_ps = eppool.tile([HPC, NLC * CHUNK], f32, tag="s_ps", bufs=2)
                nc.tensor.matmul(sl_ps[:], qbd_sb, fTl_sb[:], start=True, stop=True)
                local_state["w"] = softmax_weights(
                    sl_ps[:], n_loc, -1, False, wtag="wlsb", wbufs=1
                )

            def local_phase2():
                wl_sb = local_state["w"]
                for c in range(NLC):
                    wTl_ps = eppool.tile([CHUNK, HPC], f32, tag="wT_ps")
                    nc.tensor.transpose(
                        wTl_ps[:], wl_sb[:, c * CHUNK : (c + 1) * CHUNK],
                        ident_sb[0:HPC, 0:HPC],
                    )
                    wTl_sb = spool.tile([CHUNK, HPC], f32, tag="wTl_sb")
                    nc.scalar.copy(wTl_sb[:], wTl_ps[:])
                    out_matmul(fl_sb[:, c, :], wTl_sb[:])

            # ---- old-path (sub-BLK) node block sums ----
            def emit_old_mms(start, L):
                nch = L // CHUNK
                vt = oldpath_tiles[start]
                ps2 = apool.tile([K_MAX, 2, F], f32, tag="acco")
                done = 0
                c = 0
                while c < nch:
                    w = 2 if c + 2 <= nch else 1
                    nc.tensor.matmul(
                        ps2[:, 0:w, :], sel_sb, vt[:, c : c + w, :],
                        start=(done == 0), stop=(done + w == nch),
                    )
                    done += w
                    c += w
                f_sb = fpool.tile([K_MAX, F], f32, tag="f")
                if nch > 1:
                    nc.vector.tensor_reduce(
                        f_sb[:], ps2[:, :, :].rearrange("p c f -> p f c"),
                        axis=AX.X, op=mybir.AluOpType.add,
                    )
                else:
                    nc.scalar.copy(f_sb[:], ps2[:, 0, :])
                return f_sb

            # ---- r64 stream: sub-DMAs (r-ranges) into separate tiles ----
            stream_acc = {}   # node_i -> psum tile

            def emit_iteration(node_i, start, L, c0, subs, fillers):
                """One 2-BLK iteration of r64 node `node_i`: len(subs) sub-DMAs
                on the sync ring, then per-sub matmul groups. fillers[-1] runs
                after the DMA issues (before any matmul); fillers[j] after sub
                j's matmul group — epilogue work placed into DMA-gated gaps."""
                CC = L // BLK
                n_mm_node = ((CC + 1) // 2) * K_MAX
                base = v[start + c0 * BLK : start + (c0 + 2) * BLK, :]
                src4 = base.bitcast(f32r).rearrange(
                    "(c q r) f -> q c r f", q=CHUNK, r=K_MAX
                )
                tiles = []
                for (a, b) in subs:
                    w = b - a
                    vt = vpool.tile(
                        [CHUNK, 2, w, F], f32r, tag=f"vs{w}",
                        bufs=(5 if w >= 16 else 4),
                    )
                    nc.sync.dma_start(vt[:], src4[:, :, a:b, :])
                    tiles.append(vt)
                for fn in fillers.get(-1, []):
                    fn()
                if node_i not in stream_acc:
                    stream_acc[node_i] = apool.tile(
                        [K_MAX, 2, F], f32, tag="acc", name=f"acc{node_i}"
                    )
                ps2 = stream_acc[node_i]
                done = (c0 // 2) * K_MAX
                for j, (a, b) in enumerate(subs):
                    vt = tiles[j]
                    for r in range(a, b):
                        nc.tensor.matmul(
                            ps2[:, :, :], sel64_lhsT(r), vt[:, :, r - a, :],
                            start=(done == 0), stop=(done == n_mm_node - 1),
                        )
                        done += 1
                    for fn in fillers.get(j, []):
                        fn()

            def stream_f_add(node_i):
                ps2 = stream_acc[node_i]
                f_sb = fpool.tile([K_MAX, F], f32, tag="f")
                nc.vector.tensor_reduce(
                    f_sb[:], ps2[:, :, :].rearrange("p c f -> p f c"),
                    axis=AX.X, op=mybir.AluOpType.add,
                )
                return f_sb

            # ================= emission schedule =================
            SUBS4 = [(0, 16), (16, 32), (32, 48), (48, 64)]
            SUBS6 = [(0, 16), (16, 32), (32, 40), (40, 48), (48, 56), (56, 64)]

            iters = []   # (r64_node_idx, start, L, c0, node_last)
            for ni, (s_, L_, _d) in enumerate(r64):
                CC = L_ // BLK
                for c0 in range(0, CC, 2):
                    iters.append((ni, s_, L_, c0, c0 + 2 >= CC))
            NIT = len(iters)

            st = {}

            def fill_local1():
                local_phase1()

            def fill_smalls1():
                for si, (s_, L_, _d) in enumerate(small):
                    st[f"wsm{si}"] = tree_phase1(
                        len(big) + si, small_tiles[si][0:L_, :], L_
                    )

            def fill_old_mms():
                for oi, (s_, L_, _d) in enumerate(old):
                    st[f"fo{oi}"] = emit_old_mms(s_, L_)

            def fill_old1():
                for oi, (s_, L_, _d) in enumerate(old):
                    st[f"wo{oi}"] = tree_phase1(len(r64) + oi, st[f"fo{oi}"], K_MAX)

            def fill_local2():
                local_phase2()

            def fill_smalls2():
                for si, (s_, L_, _d) in enumerate(small):
                    tree_phase2(small_tiles[si][0:L_, :], st[f"wsm{si}"], L_)

            def fill_old2():
                for oi, (s_, L_, _d) in enumerate(old):
                    tree_phase2(st[f"fo{oi}"], st[f"wo{oi}"], K_MAX)

            def fill_r64_f_add(ni):
                def f():
                    st[f"fr{ni}"] = stream_f_add(ni)
                return f

            def fill_r64_1(ni):
                def f():
                    st[f"wr{ni}"] = tree_phase1(ni, st[f"fr{ni}"], K_MAX)
                return f

            def fill_r64_2(ni):
                def f():
                    tree_phase2(st[f"fr{ni}"], st[f"wr{ni}"], K_MAX)
                return f

            filler_maps = [dict() for _ in range(max(NIT, 1))]

            def add_fill(it, sub, fn):
                filler_maps[it].setdefault(sub, []).append(fn)

            if NIT >= 3:
                add_fill(0, 0, fill_local1)
                add_fill(0, 1, fill_smalls1)
                add_fill(0, 2, fill_old_mms)
                add_fill(0, 3, fill_old1)
                add_fill(1, 0, fill_local2)
                add_fill(1, 1, fill_smalls2)
                add_fill(1, 2, fill_old2)
            else:
                for fn in [fill_local1, fill_smalls1, fill_old_mms, fill_old1,
                           fill_local2, fill_smalls2, fill_old2]:
                    add_fill(0, 0, fn)

            # r64 node boundary epilogues: f_add + phase1 emitted BEFORE the
            # next node's first matmul (PSUM slot WAR), phase2 two sub-groups
            # later so its softmax is done; the final node's epilogue is the
            # tail.
            for it_i, (ni, s_, L_, c0, node_last) in enumerate(iters):
                if not node_last or it_i + 1 >= NIT:
                    continue
                add_fill(it_i + 1, -1, fill_r64_f_add(ni))
                add_fill(it_i + 1, -1, fill_r64_1(ni))
                add_fill(it_i + 1, 2, fill_r64_2(ni))

            for it_i, (ni, s_, L_, c0, node_last) in enumerate(iters):
                subs = SUBS6 if it_i == NIT - 1 else SUBS4
                emit_iteration(ni, s_, L_, c0, subs, filler_maps[it_i])

            if NIT:
                ni_last = iters[-1][0]
                fill_r64_f_add(ni_last)()
                fill_r64_1(ni_last)()
                fill_r64_2(ni_last)()

            # ================= final output =================
            acc_sb = spool.tile([F, HPC], f32, tag="acc_sb")
            nc.scalar.copy(acc_sb[:], out_ps[:])
            nc.sync.dma_start(o[:], acc_sb[:])

    nc.compile()
    return nc


def _make_in_maps(v_tokens, q_new, depth_proj_w, depth_temp, pos):
    big, small, r64, old = _split_nodes(pos)
    tree = big + small
    NT = len(tree)
    OFF, CB_W = _cblob_layout(NT)
    nt1 = max(NT, 1)

    sel = np.tile(np.eye(K_MAX, dtype=np.float32), (CHUNK // K_MAX, 1))
    # part j, window k (= r-block 16j+k): all-ones at window column r = 16j+k
    sel64 = np.zeros((4, CHUNK, 16 * K_MAX), np.float32)
    for j in range(4):
        for k in range(16):
            sel64[j, :, k * K_MAX + 16 * j + k] = 1.0

    wTI = np.stack(
        [np.eye(HEAD_DIM, dtype=np.float32) + depth_proj_w[d].T for (_, _, d) in tree]
    ) if NT else np.zeros((1, HEAD_DIM, HEAD_DIM), np.float32)
    tsel = np.array([depth_temp[d] for (_, _, d) in tree], np.float32) \
        if NT else np.zeros((1,), np.float32)
    msc = np.array(
        [float(K_MAX) / L if L > K_MAX else 1.0 for (_, L, _d) in tree], np.float32
    ) if NT else np.ones((1,), np.float32)
    zsc = (float(NT) / msc).astype(np.float32) if NT else np.ones((1,), np.float32)

    in_maps = []
    for c in range(N_CORES):
        q_c = q_new[0, HPC * c : HPC * (c + 1), :]          # (2, 64)
        cb = np.zeros((CHUNK, CB_W), np.float32)
        cb[:, OFF["ident"] : OFF["ident"] + CHUNK] = np.eye(CHUNK)
        for h in range(HPC):
            cb[h * HEAD_DIM : (h + 1) * HEAD_DIM, OFF["qbd"] + h] = q_c[h]
        cb[0:HEAD_DIM, OFF["qT"] : OFF["qT"] + HPC] = q_c.T
        cb[0:HPC, OFF["temps"] : OFF["temps"] + nt1] = tsel[None, :]
        cb[0:HPC, OFF["msc"] : OFF["msc"] + nt1] = msc[None, :]
        cb[0:HPC, OFF["zsc"] : OFF["zsc"] + nt1] = zsc[None, :]
        for n in range(nt1):
            cb[0:HEAD_DIM, OFF["wTI"] + n * HEAD_DIM : OFF["wTI"] + (n + 1) * HEAD_DIM] = (
                wTI[n] if NT else 0.0
            )
        im = {
            "v": np.ascontiguousarray(
                v_tokens[:, HPC * c : HPC * (c + 1), :]
            ).reshape(NTOK, F),
            "selb": np.ascontiguousarray(sel),
            "sel64": sel64,
            "cblob": cb,
        }
        in_maps.append(im)
    return in_maps


def kernel(v_tokens, q_new, depth_proj_w, depth_temp, n_tokens, _profile=False):
    global _last_results
    v_tokens = np.asarray(v_tokens, dtype=np.float32)
    q_new = np.asarray(q_new, dtype=np.float32)
    depth_proj_w = np.asarray(depth_proj_w, dtype=np.float32)
    depth_temp = np.asarray(depth_temp, dtype=np.float32)
    pos = int(n_tokens)

    nc = _build_program(pos)
    in_maps = _make_in_maps(v_tokens, q_new, depth_proj_w, depth_temp, pos)
    res = run_bass_kernel_spmd(
        nc, in_maps, core_ids=list(range(N_CORES)), trace=_profile
    )
    _last_results = res

    out = np.zeros((1, NUM_HEADS, HEAD_DIM), np.float32)
    for c in range(N_CORES):
        oT = res.results[c]["o"]                      # (128, 2)
        for h in range(HPC):
            out[0, HPC * c + h, :] = oT[h * HEAD_DIM : (h + 1) * HEAD_DIM, h]
    return out


# revision 24
# speedup vs baseline: 1.4202x; 1.0235x over previous
"""Trainium2 Bass kernel for nn_DCWTv2InferenceCache (segment-tree cached attention).

Sharding: tensor-parallel over the 16-head axis -> 8 cores x 2 heads.
Each core streams its (50000, 2*64) f32 slice of the value cache from HBM,
reduces segment-tree nodes to (64, 128) block-sums on the PE (selection-matrix
matmul accumulating in PSUM), then runs the per-node depth-projected attention
epilogue fully on-device. Output is head-sharded (2, 64) per core, gathered on
host. No cross-device communication.

v2 stream layout (vs the 127us whole-tile baseline):
  - the 25.2MB token stream goes down the Sync HWDGE ring as 14 sub-DMAs of
    ~1-2.1MB into separate tiles, so PE matmuls chase the DMA at 8-16-matmul
    granularity instead of waiting for a whole 8.4MB tile
  - constants go down the Scalar HWDGE ring and raw-token prefetch down the
    gpsimd SWDGE, so the stream's first descriptor issues right after the
    preamble instead of queueing behind 12 prefetch issues
  - the (128, 4096) r64 selection matrix is built on-device with two vector
    memsets (zero fill + stride-65 ones) instead of a 2.1MB DMA
  - epilogues are phase-split and injected into DMA-gated PE FIFO gaps; the
    mean scaling is folded into the softmax scale + Z normalization; the
    output accumulator is transposed (128, 2) so ONE output DMA suffices
"""

import math
import sys

if "/opt/trn_rl_repo" not in sys.path:
    sys.path.insert(0, "/opt/trn_rl_repo")

import numpy as np

import concourse.bass as bass
import concourse.mybir as mybir
import concourse.tile as tile
from concourse import bacc
from concourse.bass_utils import run_bass_kernel_spmd

# --- problem constants (from the reference nn.Module) ---
MAX_LEN = 65536
NUM_HEADS = 16
HEAD_DIM = 64
K_MAX = 64
LOCAL_WINDOW = 512
LOG_N = 17
LEAF_START = 2**LOG_N

N_CORES = 8
HPC = NUM_HEADS // N_CORES        # heads per core = 2
F = HPC * HEAD_DIM                # feature width per core = 128
NTOK = 50000                      # v_tokens buffer length

CHUNK = 128                       # tokens per matmul tile (partition dim)
BLK = CHUNK * K_MAX               # 8192 tokens per linear c-chunk (r64 path)
SEL64_W = K_MAX * 65              # 4160: 64 blocks of 64, viewed (64, 65)

f32 = mybir.dt.float32
f32r = mybir.dt.float32r
AF = mybir.ActivationFunctionType
AX = mybir.AxisListType

_last_results = None  # stash for test harness introspection


def _cblob_layout(NT):
    """Column offsets inside the packed (128, W) f32 constants blob."""
    nt = max(NT, 1)
    off = {}
    off["ident"] = 0
    off["qbd"] = 128
    off["qT"] = 130
    off["temps"] = 132
    off["msc"] = 132 + nt          # per-node mean_scale (folded into softmax)
    off["zsc"] = 132 + 2 * nt      # per-node NT/mean_scale (folded into Z)
    off["wTI"] = 132 + 3 * nt
    return off, 132 + 3 * nt + nt * 64


def cover_set(pos):
    """O(log n) segment-tree nodes covering prefix [0..pos-1]: (start, L, depth),
    ascending start / descending L (binary decomposition of pos)."""
    if pos <= 0:
        return []
    l, r = LEAF_START, LEAF_START + min(pos, MAX_LEN)
    out = []
    while l < r:
        if l & 1:
            d = LOG_N - int(math.floor(math.log2(l)))
            out.append(((l << d) - LEAF_START, 1 << d, d))
            l += 1
        if r & 1:
            r -= 1
            d = LOG_N - int(math.floor(math.log2(r)))
            out.append(((r << d) - LEAF_START, 1 << d, d))
        l >>= 1
        r >>= 1
    return sorted(out)


def _split_nodes(pos):
    nodes = cover_set(pos)
    big = [(s, L, d) for (s, L, d) in nodes if L > K_MAX]
    small = [(s, L, d) for (s, L, d) in nodes if L <= K_MAX]
    r64 = [(s, L, d) for (s, L, d) in big if L >= 2 * BLK and L % (2 * BLK) == 0]
    old = [(s, L, d) for (s, L, d) in big if not (L >= 2 * BLK and L % (2 * BLK) == 0)]
    return big, small, r64, old


def _build_program(pos):
    """Build the single-core Bass/Tile program (same program for all 8 cores)."""
    big, small, r64, old = _split_nodes(pos)
    # tree index order must match the host cblob: big (cover order) then small
    NT = len(big) + len(small)
    n_loc = min(pos, LOCAL_WINDOW)
    assert n_loc % CHUNK == 0, "local window must be chunk-aligned for this build"
    NLC = n_loc // CHUNK
    nt1 = max(NT, 1)

    inv_sqrt_d = 1.0 / math.sqrt(HEAD_DIM)

    nc = bacc.Bacc("TRN2", target_bir_lowering=False, debug=False)

    v = nc.dram_tensor("v", [NTOK, F], f32, kind="ExternalInput")
    selb_d = nc.dram_tensor("selb", [CHUNK, K_MAX], f32r, kind="ExternalInput")
    sel64_d = nc.dram_tensor(
        "sel64", [4, CHUNK, 16 * K_MAX], f32r, kind="ExternalInput"
    )
    CBOFF, CB_W = _cblob_layout(NT)
    cblob_d = nc.dram_tensor("cblob", [CHUNK, CB_W], f32, kind="ExternalInput")
    o = nc.dram_tensor("o", [F, HPC], f32, kind="ExternalOutput")

    with tile.TileContext(nc) as tc:
        with (
            tc.tile_pool(name="consts", bufs=1) as cpool,
            tc.tile_pool(name="vstream", bufs=5) as vpool,
            tc.tile_pool(name="fsb", bufs=3) as fpool,
            tc.tile_pool(name="ep_sb", bufs=2) as spool,
            tc.tile_pool(name="xsb", bufs=2) as xpool,
            tc.tile_pool(name="acc_ps", bufs=1, space=bass.MemorySpace.PSUM) as apool,
            tc.tile_pool(name="ep_ps", bufs=1, space=bass.MemorySpace.PSUM) as eppool,
            tc.tile_pool(name="out_ps", bufs=1, space=bass.MemorySpace.PSUM) as opool,
        ):
            # ---- constants on the Scalar HWDGE ring (2 issues only, so the
            # scalar compute queue is free early for the scale/qd chains);
            # raw-token prefetch on the gpsimd SWDGE; the big token stream owns
            # the Sync HWDGE ring so its first descriptor issues immediately.
            cb = cpool.tile([CHUNK, CB_W], f32)
            nc.scalar.dma_start(cb[:], cblob_d[:])
            ident_sb = cb[:, CBOFF["ident"] : CBOFF["ident"] + CHUNK]
            qbd_sb = cb[:, CBOFF["qbd"] : CBOFF["qbd"] + HPC]
            qT_sb = cb[0:HEAD_DIM, CBOFF["qT"] : CBOFF["qT"] + HPC]
            temps2_sb = cb[0:HPC, CBOFF["temps"] : CBOFF["temps"] + nt1]
            msc_sb = cb[0:HPC, CBOFF["msc"] : CBOFF["msc"] + nt1]
            zsc_sb = cb[0:HPC, CBOFF["zsc"] : CBOFF["zsc"] + nt1]

            def wTI_slice(n):
                c0 = CBOFF["wTI"] + n * HEAD_DIM
                return cb[0:HEAD_DIM, c0 : c0 + HEAD_DIM]

            # ---- r64 selection matrix, 4 part-tiles of 16 r-blocks each so
            # the first stream matmuls are gated only by part 0 (0.52MB).
            # Scalar-ring order below is by first-use time.
            sel64_t = []
            for j in range(4):
                s64 = cpool.tile(
                    [CHUNK, 16 * K_MAX], f32r, name=f"sel64_{j}", tag=f"sel64_{j}"
                )
                sel64_t.append(s64)
            nc.scalar.dma_start(sel64_t[0][:], sel64_d[0])
            nc.scalar.dma_start(sel64_t[1][:], sel64_d[1])

            def sel64_lhsT(r):
                j, k = r // 16, r % 16
                return sel64_t[j][:, k * K_MAX : (k + 1) * K_MAX]

            # eye fold matrix for the old path (tokens mod 64 within a chunk)
            sel_t = cpool.tile([CHUNK, K_MAX], f32r)
            nc.scalar.dma_start(sel_t[:], selb_d[:])
            sel_sb = sel_t[:]

            # old-path nodes (one DMA each) + small tail nodes + local window
            oldpath_tiles = {}
            for (start_b, L_b, _d) in old:
                nch_b = L_b // CHUNK
                vo = cpool.tile(
                    [CHUNK, nch_b, F], f32r, name=f"vo{start_b}", tag=f"vo{start_b}"
                )
                src_o = v[start_b : start_b + L_b, :].bitcast(f32r)
                nc.scalar.dma_start(
                    vo[:], src_o.rearrange("(c p) f -> p c f", p=CHUNK)
                )
                oldpath_tiles[start_b] = vo
            small_tiles = []
            for si, (start_s, L_s, _d) in enumerate(small):
                fsm = cpool.tile([K_MAX, F], f32, name=f"fsm{si}", tag=f"fsm{si}")
                nc.scalar.dma_start(fsm[0:L_s, :], v[start_s : start_s + L_s, :])
                small_tiles.append(fsm)
            lstart = pos - n_loc
            fl_sb = cpool.tile([CHUNK, NLC, F], f32)
            nc.scalar.dma_start(
                fl_sb[:],
                v[lstart : lstart + n_loc, :].rearrange("(c p) f -> p c f", p=CHUNK),
            )
            nc.scalar.dma_start(sel64_t[2][:], sel64_d[2])
            nc.scalar.dma_start(sel64_t[3][:], sel64_d[3])

            # ---- per-node softmax scales: msc/((softplus(t)+1e-6)*sqrt(D)) ----
            et_sb = cpool.tile([HPC, nt1], f32)
            nc.scalar.activation(et_sb[:], temps2_sb, AF.Exp)
            sp_sb = cpool.tile([HPC, nt1], f32)
            nc.scalar.activation(sp_sb[:], et_sb[:], AF.Ln, bias=1.0)  # softplus
            u_sb = cpool.tile([HPC, nt1], f32)
            nc.scalar.mul(u_sb[:], sp_sb, math.sqrt(HEAD_DIM))
            nc.vector.tensor_scalar_add(u_sb[:], u_sb[:], 1e-6 * math.sqrt(HEAD_DIM))
            r0_sb = cpool.tile([HPC, nt1], f32)
            nc.vector.reciprocal(r0_sb[:], u_sb[:])
            rs_sb = cpool.tile([HPC, nt1], f32)
            nc.vector.tensor_mul(rs_sb[:], r0_sb[:], msc_sb)   # fold mean scale
            ns_sb = cpool.tile([HPC, nt1], f32)
            nc.scalar.mul(ns_sb[:], rs_sb[:], -1.0)

            # ---- all tree-node q_depth projections upfront (block-diag);
            # PE runs these while the first stream sub-DMA is in flight.
            qd_all = cpool.tile([2 * HEAD_DIM, nt1, HPC], f32)
            nc.vector.memset(qd_all[:], 0.0)
            for n in range(NT):
                qd_ps = eppool.tile([2 * HEAD_DIM, HPC], f32, tag="qd_ps")
                nc.tensor.matmul(
                    qd_ps[0:HEAD_DIM, 0:1],
                    wTI_slice(n), qT_sb[:, 0:1], start=True, stop=True,
                )
                nc.tensor.matmul(
                    qd_ps[HEAD_DIM : 2 * HEAD_DIM, 1:2],
                    wTI_slice(n), qT_sb[:, 1:2], start=True, stop=True,
                )
                nc.scalar.copy(qd_all[0:HEAD_DIM, n, 0:1], qd_ps[0:HEAD_DIM, 0:1])
                nc.scalar.copy(
                    qd_all[HEAD_DIM : 2 * HEAD_DIM, n, 1:2],
                    qd_ps[HEAD_DIM : 2 * HEAD_DIM, 1:2],
                )

            # ---- cross-node TRANSPOSED output accumulator (128, 2) PSUM ----
            out_ps = opool.tile([F, HPC], f32)
            n_out_mm = NT + NLC
            out_mm = [0]

            def out_matmul(f_sb_ap, wT_sb_ap):
                # out_T[(h d), h'] += sum_k f[k, (h d)] * wT[k, h']
                nc.tensor.matmul(
                    out_ps[:], f_sb_ap, wT_sb_ap,
                    start=(out_mm[0] == 0), stop=(out_mm[0] == n_out_mm - 1),
                )
                out_mm[0] += 1

            def softmax_weights(s_ps_ap, K, node_i, is_tree, wtag="wsb", wbufs=6):
                """softmax over K free-dim entries of (2, K) logits (pre-scale);
                tree nodes fold mean_scale/NT into the weights via zsc."""
                smax = xpool.tile([HPC, 1], f32, tag="smax" + wtag, bufs=wbufs)
                nc.vector.reduce_max(smax[:], s_ps_ap, axis=AX.X)
                biast = xpool.tile([HPC, 1], f32, tag="biast" + wtag, bufs=wbufs)
                ebd = xpool.tile([HPC, K], f32, tag="esb" + wtag, bufs=wbufs)
                zt = xpool.tile([HPC, 1], f32, tag="zt" + wtag, bufs=wbufs)
                if is_tree:
                    nc.vector.tensor_scalar_mul(
                        biast[:], smax[:], ns_sb[:, node_i : node_i + 1]
                    )
                    nc.scalar.activation(
                        ebd[:], s_ps_ap, AF.Exp,
                        bias=biast[:], scale=rs_sb[:, node_i : node_i + 1],
                        accum_out=zt[:],
                    )
                    zs = xpool.tile([HPC, 1], f32, tag="zs" + wtag, bufs=wbufs)
                    nc.vector.tensor_scalar_mul(
                        zs[:], zt[:], zsc_sb[:, node_i : node_i + 1]
                    )
                    zt = zs
                else:
                    nc.scalar.mul(biast[:], smax[:], -inv_sqrt_d)
                    nc.scalar.activation(
                        ebd[:], s_ps_ap, AF.Exp, bias=biast[:], scale=inv_sqrt_d,
                        accum_out=zt[:],
                    )
                rz = xpool.tile([HPC, 1], f32, tag="rz" + wtag, bufs=wbufs)
                nc.vector.reciprocal(rz[:], zt[:])
                w_sb = xpool.tile([HPC, K], f32, tag=wtag, bufs=wbufs)
                nc.vector.tensor_scalar_mul(w_sb[:], ebd[:], rz[:])
                return w_sb

            # ---- phase-split tree epilogue. PSUM tags are shared with the
            # local-window epilogue (padded allocs) to stay within 8 banks:
            # acc(1) acco(1) qd(1) fT(1) s(2) wT(1) out(1) = 8.
            def tree_phase1(node_i, f_sb_ap, K):
                fT_ps = eppool.tile([F, CHUNK], f32, tag="fT_ps")
                nc.tensor.transpose(fT_ps[:, 0:K], f_sb_ap, ident_sb[0:K, 0:K])
                fT_sb = spool.tile([F, K_MAX], f32, tag="fT_sb")
                nc.scalar.copy(fT_sb[:, 0:K], fT_ps[:, 0:K])
                s_ps = eppool.tile([HPC, NLC * CHUNK], f32, tag="s_ps", bufs=2)
                nc.tensor.matmul(
                    s_ps[:, 0:K], qd_all[:, node_i, :], fT_sb[:, 0:K],
                    start=True, stop=True,
                )
                return softmax_weights(s_ps[:, 0:K], K, node_i, True)

            def tree_phase2(f_sb_ap, w_sb, K):
                wT_ps = eppool.tile([CHUNK, HPC], f32, tag="wT_ps")
                nc.tensor.transpose(wT_ps[0:K, :], w_sb[:], ident_sb[0:HPC, 0:HPC])
                wT_sb = spool.tile([K_MAX, HPC], f32, tag="wT_sb")
                nc.scalar.copy(wT_sb[0:K, :], wT_ps[0:K, :])
                out_matmul(f_sb_ap, wT_sb[0:K, :])

            # ---- local window epilogue, phase-split the same way ----
            local_state = {}

            def local_phase1():
                fTl_sb = spool.tile([F, NLC * CHUNK], f32, tag="fTl_sb", bufs=1)
                for c in range(NLC):
                    fTl_ps = eppool.tile([F, CHUNK], f32, tag="fT_ps")
                    nc.tensor.transpose(fTl_ps[:], fl_sb[:, c, :], ident_sb[:])
                    nc.scalar.copy(
                        fTl_sb[:, c * CHUNK : (c + 1) * CHUNK], fTl_ps[:]
                    )
                sl_ps = eppool.tile([HPC, NLC * CHUNK], f32, tag="s_ps", bufs=2)
                nc.tensor.matmul(sl_ps[:], qbd_sb, fTl_sb[:], start=True, stop=True)
                local_state["w"] = softmax_weights(
                    sl_ps[:], n_loc, -1, False, wtag="wlsb", wbufs=1
                )

            def local_phase2():
                wl_sb = local_state["w"]
                for c in range(NLC):
                    wTl_ps = eppool.tile([CHUNK, HPC], f32, tag="wT_ps")
                    nc.tensor.transpose(
                        wTl_ps[:], wl_sb[:, c * CHUNK : (c + 1) * CHUNK],
                        ident_sb[0:HPC, 0:HPC],
                    )
                    wTl_sb = spool.tile([CHUNK, HPC], f32, tag="wTl_sb")
                    nc.scalar.copy(wTl_sb[:], wTl_ps[:])
                    out_matmul(fl_sb[:, c, :], wTl_sb[:])

            # ---- old-path (sub-BLK) node block sums ----
            def emit_old_mms(start, L):
                nch = L // CHUNK
                vt = oldpath_tiles[start]
                ps2 = apool.tile([K_MAX, 2, F], f32, tag="acco")
                done = 0
                c = 0
                while c < nch:
                    w = 2 if c + 2 <= nch else 1
                    nc.tensor.matmul(
                        ps2[:, 0:w, :], sel_sb, vt[:, c : c + w, :],
                        start=(done == 0), stop=(done + w == nch),
                    )
                    done += w
                    c += w
                f_sb = fpool.tile([K_MAX, F], f32, tag="f")
                if nch > 1:
                    nc.vector.tensor_reduce(
                        f_sb[:], ps2[:, :, :].rearrange("p c f -> p f c"),
                        axis=AX.X, op=mybir.AluOpType.add,
                    )
                else:
                    nc.scalar.copy(f_sb[:], ps2[:, 0, :])
                return f_sb

            # ---- r64 stream: sub-DMAs (r-ranges) into separate tiles ----
            stream_acc = {}   # node_i -> psum tile

            def emit_iteration(node_i, start, L, c0, subs, fillers):
                """One 2-BLK iteration of r64 node `node_i`: len(subs) sub-DMAs
                on the sync ring, then per-sub matmul groups. fillers[-1] runs
                after the DMA issues (before any matmul); fillers[j] after sub
                j's matmul group — epilogue work placed into DMA-gated gaps."""
                CC = L // BLK
                n_mm_node = ((CC + 1) // 2) * K_MAX
                base = v[start + c0 * BLK : start + (c0 + 2) * BLK, :]
                src4 = base.bitcast(f32r).rearrange(
                    "(c q r) f -> q c r f", q=CHUNK, r=K_MAX
                )
                tiles = []
                for (a, b) in subs:
                    w = b - a
                    vt = vpool.tile(
                        [CHUNK, 2, w, F], f32r, tag=f"vs{w}",
                        bufs=(6 if w >= 16 else 4),
                    )
                    nc.sync.dma_start(vt[:], src4[:, :, a:b, :])
                    tiles.append(vt)
                for fn in fillers.get(-1, []):
                    fn()
                if node_i not in stream_acc:
                    stream_acc[node_i] = apool.tile(
                        [K_MAX, 2, F], f32, tag="acc", name=f"acc{node_i}"
                    )
                ps2 = stream_acc[node_i]
                done = (c0 // 2) * K_MAX
                for j, (a, b) in enumerate(subs):
                    vt = tiles[j]
                    for r in range(a, b):
                        nc.tensor.matmul(
                            ps2[:, :, :], sel64_lhsT(r), vt[:, :, r - a, :],
                            start=(done == 0), stop=(done == n_mm_node - 1),
                        )
                        done += 1
                    for fn in fillers.get(j, []):
                        fn()

            def stream_f_add(node_i):
                ps2 = stream_acc[node_i]
                f_sb = fpool.tile([K_MAX, F], f32, tag="f")
                nc.vector.tensor_reduce(
                    f_sb[:], ps2[:, :, :].rearrange("p c f -> p f c"),
                    axis=AX.X, op=mybir.AluOpType.add,
                )
                return f_sb

            # ================= emission schedule =================
            SUBS4 = [(0, 16), (16, 32), (32, 48), (48, 64)]
            SUBS6 = [(0, 16), (16, 32), (32, 40), (40, 48), (48, 56), (56, 64)]

            iters = []   # (r64_node_idx, start, L, c0, node_last)
            for ni, (s_, L_, _d) in enumerate(r64):
                CC = L_ // BLK
                for c0 in range(0, CC, 2):
                    iters.append((ni, s_, L_, c0, c0 + 2 >= CC))
            NIT = len(iters)

            st = {}

            def fill_local1():
                local_phase1()

            def fill_smalls1():
                for si, (s_, L_, _d) in enumerate(small):
                    st[f"wsm{si}"] = tree_phase1(
                        len(big) + si, small_tiles[si][0:L_, :], L_
                    )

            def fill_old_mms():
                for oi, (s_, L_, _d) in enumerate(old):
                    st[f"fo{oi}"] = emit_old_mms(s_, L_)

            def fill_old1():
                for oi, (s_, L_, _d) in enumerate(old):
                    st[f"wo{oi}"] = tree_phase1(len(r64) + oi, st[f"fo{oi}"], K_MAX)

            def fill_local2():
                local_phase2()

            def fill_smalls2():
                for si, (s_, L_, _d) in enumerate(small):
                    tree_phase2(small_tiles[si][0:L_, :], st[f"wsm{si}"], L_)

            def fill_old2():
                for oi, (s_, L_, _d) in enumerate(old):
                    tree_phase2(st[f"fo{oi}"], st[f"wo{oi}"], K_MAX)

            def fill_r64_f_add(ni):
                def f():
                    st[f"fr{ni}"] = stream_f_add(ni)
                return f

            def fill_r64_1(ni):
                def f():
                    st[f"wr{ni}"] = tree_phase1(ni, st[f"fr{ni}"], K_MAX)
                return f

            def fill_r64_2(ni):
                def f():
                    tree_phase2(st[f"fr{ni}"], st[f"wr{ni}"], K_MAX)
                return f

            filler_maps = [dict() for _ in range(max(NIT, 1))]

            def add_fill(it, sub, fn):
                filler_maps[it].setdefault(sub, []).append(fn)

            if NIT >= 3:
                add_fill(0, 0, fill_local1)
                add_fill(0, 1, fill_smalls1)
                add_fill(0, 2, fill_old_mms)
                add_fill(0, 3, fill_old1)
                add_fill(1, 0, fill_local2)
                add_fill(1, 1, fill_smalls2)
                add_fill(1, 2, fill_old2)
            else:
                for fn in [fill_local1, fill_smalls1, fill_old_mms, fill_old1,
                           fill_local2, fill_smalls2, fill_old2]:
                    add_fill(0, 0, fn)

            # r64 node boundary epilogues: f_add + phase1 emitted BEFORE the
            # next node's first matmul (PSUM slot WAR), phase2 two sub-groups
            # later so its softmax is done; the final node's epilogue is the
            # tail.
            # f_add must precede the next node's first matmul in emission order
            # (PSUM slot WAR) but runs on the vector queue, so it cannot stall
            # the PE FIFO; phase1's PE ops go after the next sub-group instead.
            for it_i, (ni, s_, L_, c0, node_last) in enumerate(iters):
                if not node_last or it_i + 1 >= NIT:
                    continue
                add_fill(it_i + 1, -1, fill_r64_f_add(ni))
                add_fill(it_i + 1, 0, fill_r64_1(ni))
                add_fill(it_i + 1, 2, fill_r64_2(ni))

            for it_i, (ni, s_, L_, c0, node_last) in enumerate(iters):
                subs = SUBS6 if it_i == NIT - 1 else SUBS4
                emit_iteration(ni, s_, L_, c0, subs, filler_maps[it_i])

            if NIT:
                ni_last = iters[-1][0]
                fill_r64_f_add(ni_last)()
                fill_r64_1(ni_last)()
                fill_r64_2(ni_last)()

            # ================= final output =================
            acc_sb = spool.tile([F, HPC], f32, tag="acc_sb")
            nc.scalar.copy(acc_sb[:], out_ps[:])
            nc.sync.dma_start(o[:], acc_sb[:])

    nc.compile()
    return nc


def _make_in_maps(v_tokens, q_new, depth_proj_w, depth_temp, pos):
    big, small, r64, old = _split_nodes(pos)
    tree = big + small
    NT = len(tree)
    OFF, CB_W = _cblob_layout(NT)
    nt1 = max(NT, 1)

    sel = np.tile(np.eye(K_MAX, dtype=np.float32), (CHUNK // K_MAX, 1))
    # part j, window k (= r-block 16j+k): all-ones at window column r = 16j+k
    sel64 = np.zeros((4, CHUNK, 16 * K_MAX), np.float32)
    for j in range(4):
        for k in range(16):
            sel64[j, :, k * K_MAX + 16 * j + k] = 1.0

    wTI = np.stack(
        [np.eye(HEAD_DIM, dtype=np.float32) + depth_proj_w[d].T for (_, _, d) in tree]
    ) if NT else np.zeros((1, HEAD_DIM, HEAD_DIM), np.float32)
    tsel = np.array([depth_temp[d] for (_, _, d) in tree], np.float32) \
        if NT else np.zeros((1,), np.float32)
    msc = np.array(
        [float(K_MAX) / L if L > K_MAX else 1.0 for (_, L, _d) in tree], np.float32
    ) if NT else np.ones((1,), np.float32)
    zsc = (float(NT) / msc).astype(np.float32) if NT else np.ones((1,), np.float32)

    in_maps = []
    for c in range(N_CORES):
        q_c = q_new[0, HPC * c : HPC * (c + 1), :]          # (2, 64)
        cb = np.zeros((CHUNK, CB_W), np.float32)
        cb[:, OFF["ident"] : OFF["ident"] + CHUNK] = np.eye(CHUNK)
        for h in range(HPC):
            cb[h * HEAD_DIM : (h + 1) * HEAD_DIM, OFF["qbd"] + h] = q_c[h]
        cb[0:HEAD_DIM, OFF["qT"] : OFF["qT"] + HPC] = q_c.T
        cb[0:HPC, OFF["temps"] : OFF["temps"] + nt1] = tsel[None, :]
        cb[0:HPC, OFF["msc"] : OFF["msc"] + nt1] = msc[None, :]
        cb[0:HPC, OFF["zsc"] : OFF["zsc"] + nt1] = zsc[None, :]
        for n in range(nt1):
            cb[0:HEAD_DIM, OFF["wTI"] + n * HEAD_DIM : OFF["wTI"] + (n + 1) * HEAD_DIM] = (
                wTI[n] if NT else 0.0
            )
        im = {
            "v": np.ascontiguousarray(
                v_tokens[:, HPC * c : HPC * (c + 1), :]
            ).reshape(NTOK, F),
            "selb": np.ascontiguousarray(sel),
            "sel64": sel64,
            "cblob": cb,
        }
        in_maps.append(im)
    return in_maps


def kernel(v_tokens, q_new, depth_proj_w, depth_temp, n_tokens, _profile=False):
    global _last_results
    v_tokens = np.asarray(v_tokens, dtype=np.float32)
    q_new = np.asarray(q_new, dtype=np.float32)
    depth_proj_w = np.asarray(depth_proj_w, dtype=np.float32)
    depth_temp = np.asarray(depth_temp, dtype=np.float32)
    pos = int(n_tokens)

    nc = _build_program(pos)
    in_maps = _make_in_maps(v_tokens, q_new, depth_proj_w, depth_temp, pos)
    res = run_bass_kernel_spmd(
        nc, in_maps, core_ids=list(range(N_CORES)), trace=_profile
    )
    _last_results = res

    out = np.zeros((1, NUM_HEADS, HEAD_DIM), np.float32)
    for c in range(N_CORES):
        oT = res.results[c]["o"]                      # (128, 2)
        for h in range(HPC):
            out[0, HPC * c + h, :] = oT[h * HEAD_DIM : (h + 1) * HEAD_DIM, h]
    return out
